# revision 84
# baseline (speedup 1.0000x reference)
"""ChoiceAttention Trainium2 kernel.

Math (per batch item b, per "retain" iteration a over the 5 options):
    q_a = opt_a @ W                              (s, h)
    S_ak[p, r] = q_a[p, :] . opt_k[r, :]         for the 4 options k != a
    w_ak = softmax over k of (S_ak + bias)       (bias cancels: softmax is
                                                  shift-invariant over k)
    out += sum_k w_ak @ opt_k
final out /= 2.

Sharding: data-parallel over batch across 8 NeuronCores (4 items each),
W replicated. No collectives; host concatenates the per-core outputs.

Layout strategy per core / batch item:
    nat_k : opt_k natural layout      (128p, 2 sc, 1024h)  - DMA'd in
    x_k   : opt_k transposed (h-major)(128p, 8 hc, 256s)   - PE transposes
    q_a^T : h-major q                 (128p, 8 hc, 256s)   - matmul(W, x_a)
    S_ak^T: scores transposed         (128p, 2 rc, 256p)   - matmul(x_k, q_a^T)
    softmax over the four k tiles elementwise (max-subtract, exp, recip)
    out   : accumulated in 4 PSUM banks over all 40 (a,k,rc) matmul groups
All matmuls run as float32r (full PE rate, fp32 storage).
"""

import numpy as np

B, S, H = 32, 256, 1024
NCORES = 8
BPC = B // NCORES  # batch items per core
P = 128
HC = H // P  # 8 h-chunks
SC = S // P  # 2 s-chunks
NOPT = 5

_CACHE: dict = {}


def _build_bass(reps: int = 1, cfg: dict | None = None):
    cfg = dict(cfg or {})
    NAT_BUFS = cfg.get("nat_bufs", 2 * NOPT)
    XT_BUFS = cfg.get("xt_bufs", 2 * NOPT)
    WS_BUFS = cfg.get("ws_bufs", 5)
    E_BUFS = cfg.get("e_bufs", 5)
    OSB_BUFS = cfg.get("osb_bufs", 2)
    GP_SUB = cfg.get("gp_sub", False)
    PSM = cfg.get("ps_misc", 2)
    PSS = cfg.get("ps_s", 4)
    PSO = cfg.get("ps_o", 2)
    Q_N128 = cfg.get("q_n128", False)   # contiguous moving operand, N=128
    # dma_start_transpose SBUF->SBUF produces wrong results on real HW (fine
    # in CoreSim) -- PE transposes are the verified path
    PE_TRANSPOSE = cfg.get("pe_transpose", True)
    SM_SBUF = cfg.get("sm_sbuf", False)  # softmax reads SBUF copies, not PSUM
    from contextlib import ExitStack

    import concourse.mybir as mybir
    import concourse.tile as tile
    from concourse import bacc
    from concourse.masks import make_identity

    FP32 = mybir.dt.float32
    F32R = mybir.dt.float32r
    BF16 = mybir.dt.bfloat16
    AF = mybir.ActivationFunctionType

    nc = bacc.Bacc(debug=False)

    opt_d = [
        nc.dram_tensor(f"option{i + 1}", (BPC, S, H), F32R, kind="ExternalInput")
        for i in range(NOPT)
    ]
    w_d = nc.dram_tensor("W", (H, H), F32R, kind="ExternalInput")
    out_d = nc.dram_tensor("out", (BPC, S, H), FP32, kind="ExternalOutput")

    with ExitStack() as ctx:
        tc = ctx.enter_context(tile.TileContext(nc))
        const = ctx.enter_context(tc.tile_pool(name="const", bufs=1))
        natbf = ctx.enter_context(tc.tile_pool(name="nat", bufs=NAT_BUFS))
        natf = ctx.enter_context(tc.tile_pool(name="natf", bufs=3))
        xp = ctx.enter_context(tc.tile_pool(name="xt", bufs=XT_BUFS))
        qp = ctx.enter_context(tc.tile_pool(name="qq", bufs=3))
        sp = ctx.enter_context(tc.tile_pool(name="ss", bufs=6))
        ep = ctx.enter_context(tc.tile_pool(name="ee", bufs=E_BUFS))
        mp_ = ctx.enter_context(tc.tile_pool(name="mm", bufs=2))
        zp = ctx.enter_context(tc.tile_pool(name="zz", bufs=2))
        rp = ctx.enter_context(tc.tile_pool(name="rr", bufs=2))
        wsp = ctx.enter_context(tc.tile_pool(name="wsum", bufs=WS_BUFS))
        op_ = ctx.enter_context(tc.tile_pool(name="osb", bufs=OSB_BUFS))
        ps_misc = ctx.enter_context(tc.tile_pool(name="ps_misc", bufs=PSM, space="PSUM"))
        ps_s = ctx.enter_context(tc.tile_pool(name="ps_s", bufs=PSS, space="PSUM"))
        ps_o = ctx.enter_context(tc.tile_pool(name="ps_o", bufs=PSO, space="PSUM"))

        w_sb = const.tile([P, HC, H], BF16)
        if PE_TRANSPOSE:
            ident_f = const.tile([P, P], FP32)
            make_identity(nc, ident_f)
            ident = const.tile([P, P], BF16)
            nc.vector.tensor_copy(out=ident, in_=ident_f)

        from contextlib import nullcontext

        loop_cm = tc.For_i(0, reps, 1) if reps > 1 else nullcontext()
        with loop_cm:
            # cross-batch carried prefetch: nat/x for all options, q for 0/1,
            # flush = the previous item's deferred pass-1 scale-copies + DMA
            carry = {"nat": {}, "x": {}, "q": {}, "flush": None}

            def load_nat(b, k):
                # fp32 on the hwdge rings (swdge cast-DMAs are ~3x slower and
                # hog Pool's queue), bf16 convert alternating ACT/Pool
                nf = natf.tile([P, SC, H], F32R, tag="natf", name=f"natf_{b}_{k}")
                src = opt_d[k].ap()[b].rearrange("(sc p) h -> p sc h", p=P)
                ring = nc.sync if k % 2 == 0 else nc.scalar
                ring.dma_start(out=nf, in_=src)
                nk = natbf.tile([P, SC, H], BF16, tag="nat", name=f"nat_{b}_{k}")
                if k % 2 == 0:
                    nc.scalar.copy(out=nk, in_=nf)
                else:
                    nc.gpsimd.tensor_copy(out=nk, in_=nf)
                return nk

            def transpose_opt(b, k, nk):
                # Layout [P, sc, hc, s]: the destination of each DMA transpose
                # must be CONTIGUOUS per partition -- a non-contiguous dest
                # gives wrong results on real HW.
                xk = xp.tile([P, SC, HC, P], BF16, tag="xt", name=f"x_{b}_{k}")
                if PE_TRANSPOSE:
                    for j in range(HC // 2):  # pairs of h-chunks -> one bank
                        pt = ps_misc.tile([P, 4, P], BF16, tag="ps_misc",
                                          name=f"pt_{b}_{k}_{j}")
                        for sc in range(SC):
                            for d in range(2):
                                nc.tensor.transpose(
                                    out=pt[:, 2 * sc + d, :],
                                    in_=nk[:, sc, (2 * j + d) * P : (2 * j + d + 1) * P],
                                    identity=ident,
                                )
                        nc.scalar.copy(out=xk[:, :, 2 * j : 2 * j + 2, :], in_=pt)
                else:
                    # 16-bit DMA-XBAR transpose on the sync ring (no PE time,
                    # no PSUM, no copies)
                    for sc in range(SC):
                        nc.sync.dma_start_transpose(
                            out=xk[:, sc, :, :], in_=nk[:, sc, :]
                        )
                return xk

            for b in range(BPC):
                if b == 0:
                    # W: four fp32 quarter-DMAs alternating between the two
                    # hwdge rings (parallel transfers), bf16 converts split
                    # across the startup-idle DVE/ACT engines
                    w_src = w_d.ap().rearrange("(kc p) h -> p kc h", p=P)
                    for quarter in range(4):
                        sl = slice(2 * quarter, 2 * quarter + 2)
                        wst = natf.tile([P, SC, H], F32R, tag="natf",
                                        name=f"wst_{quarter}")
                        ring = nc.sync if quarter % 2 == 0 else nc.scalar
                        ring.dma_start(out=wst, in_=w_src[:, sl, :])
                        if quarter % 2 == 0:
                            nc.vector.tensor_copy(out=w_sb[:, sl, :], in_=wst)
                        else:
                            nc.scalar.copy(out=w_sb[:, sl, :], in_=wst)
                # ---- load options; carried from prev item's prefetch ----
                nat = []
                for k in range(NOPT):
                    nat.append(carry["nat"].get(k) or load_nat(b, k))
                x = [None] * NOPT
                for k in range(NOPT):
                    x[k] = carry["x"].get(k) or transpose_opt(b, k, nat[k])
                carry["nat"] = {}
                carry["x"] = {}

                # ---- q_a^T = W^T @ opt_a^T, pipelined with the a-loop ----
                q = [None] * NOPT
                for kq, qt in carry["q"].items():
                    q[kq] = qt
                carry["q"] = {}

                def q_group_thunks(a, xs=None, bq=None, qdst=None):
                    """Create q_a's tile and return one thunk per PSUM group;
                    callers interleave the thunks between score groups so the
                    PSUM-copy round trip is off PE's critical path."""
                    xs = x if xs is None else xs
                    bq = b if bq is None else bq
                    qt = qp.tile([P, HC, S], BF16, tag="qq", name=f"q_{bq}_{a}")
                    if qdst is None:
                        q[a] = qt
                    else:
                        qdst[a] = qt

                    def mk(half):
                        def f():
                            pq = ps_misc.tile([P, 2, S], FP32, tag="ps_misc",
                                              name=f"pq_{bq}_{a}_{half}")
                            for d in range(2):
                                mc = 2 * half + d
                                if Q_N128:
                                    for sc in range(SC):
                                        for kc in range(HC):
                                            nc.tensor.matmul(
                                                pq[:, d, sc * P : (sc + 1) * P],
                                                w_sb[:, kc, mc * P : (mc + 1) * P],
                                                xs[a][:, sc, kc, :],
                                                start=(kc == 0),
                                                stop=(kc == HC - 1),
                                            )
                                else:
                                    for kc in range(HC):
                                        nc.tensor.matmul(
                                            pq[:, d, :],
                                            w_sb[:, kc, mc * P : (mc + 1) * P],
                                            xs[a][:, :, kc, :],
                                            start=(kc == 0),
                                            stop=(kc == HC - 1),
                                        )
                            # PSUM can only be read by ACT/DVE; ACT does the
                            # copies, latency hidden by score interleaving
                            nc.scalar.copy(
                                out=qt[:, 2 * half : 2 * half + 2, :], in_=pq)
                        return f

                    return [mk(h) for h in range(HC // 2)]

                def emit_q(a, xs=None, bq=None, qdst=None):
                    for t in q_group_thunks(a, xs, bq, qdst):
                        t()

                def emit_scores(a, qthunks=()):
                    # leaves the 4 score tiles in PSUM; softmax reads them
                    # there (no copy) and the subs free the banks. One q-group
                    # of the NEXT q rides between score groups.
                    s_ps = []
                    ki = [k for k in range(NOPT) if k != a]
                    for idx, k in enumerate(ki):
                        if idx < len(qthunks):
                            qthunks[idx]()
                        st = ps_s.tile([P, SC, S], FP32, tag="ps_s",
                                       name=f"st_{b}_{a}_{k}")
                        for rc in range(SC):
                            for hc in range(HC):
                                nc.tensor.matmul(
                                    st[:, rc, :],
                                    x[k][:, rc, hc, :],
                                    q[a][:, hc, :],
                                    start=(hc == 0),
                                    stop=(hc == HC - 1),
                                )
                        if SM_SBUF:
                            ssb = sp.tile([P, SC, S], FP32, tag="ss",
                                          name=f"ssb_{b}_{a}_{k}")
                            nc.scalar.copy(out=ssb, in_=st)
                            s_ps.append(ssb)
                        else:
                            s_ps.append(st)
                    return s_ps

                # wsum[k] accumulates sum_a softmax_weight(a, k): the output
                # matmul collapses to sum_k wsum_k @ opt_k (4x fewer matmuls)
                wsum = [None] * NOPT

                def emit_softmax(a, s_ps, tail_split=False):
                    # max chain reads score PSUM (ACT seeds the copy, DVE maxes
                    # with one PSUM operand each); subs free the score banks
                    m = mp_.tile([P, SC, S], FP32, tag="mm", name=f"m_{b}_{a}")
                    if SM_SBUF:
                        nc.vector.tensor_max(m, s_ps[0], s_ps[1])
                        nc.vector.tensor_max(m, m, s_ps[2])
                        nc.vector.tensor_max(m, m, s_ps[3])
                    else:
                        ma = mp_.tile([P, SC, S], FP32, tag="mm",
                                      name=f"ma_{b}_{a}")
                        nc.scalar.copy(out=ma, in_=s_ps[0])
                        nc.vector.tensor_max(m, ma, s_ps[1])
                        nc.vector.tensor_max(m, m, s_ps[2])
                        nc.vector.tensor_max(m, m, s_ps[3])
                    e = []
                    for k4 in range(4):
                        d = sp.tile([P, SC, S], FP32, tag="ss",
                                    name=f"d_{b}_{a}_{k4}")
                        nc.vector.tensor_sub(d, s_ps[k4], m)
                        ek = ep.tile([P, SC, S], F32R, tag="ee",
                                     name=f"e_{b}_{a}_{k4}")
                        nc.scalar.activation(out=ek, in_=d, func=AF.Exp)
                        e.append(ek)
                    z = zp.tile([P, SC, S], FP32, tag="zz", name=f"z_{b}_{a}")
                    rcp = rp.tile([P, SC, S], FP32, tag="rr", name=f"r_{b}_{a}")
                    if tail_split:
                        nc.gpsimd.tensor_add(z, e[0], e[1])
                    else:
                        nc.vector.tensor_add(z, e[0], e[1])
                    nc.vector.tensor_add(rcp, e[2], e[3])
                    nc.vector.tensor_add(z, z, rcp)
                    nc.vector.reciprocal(rcp, z)
                    ks = [k for k in range(NOPT) if k != a]
                    final = a == NOPT - 1
                    order = list(range(len(ks)))
                    if a == NOPT - 2:
                        # update wsum for the last option first: its early
                        # out-matmuls (below) wait on it
                        order.sort(key=lambda i: 0 if ks[i] == NOPT - 1 else 1 + i)
                    for idx, (k4, k) in enumerate((i, ks[i]) for i in order):
                        eng = nc.gpsimd if (tail_split and idx % 2) else nc.vector
                        if wsum[k] is None:
                            wk = wsp.tile([P, SC, S], BF16, tag="wsum",
                                          name=f"ws_{b}_{k}")
                            eng.tensor_mul(wk, e[k4], rcp)
                            wsum[k] = wk
                        else:
                            eng.tensor_mul(e[k4], e[k4], rcp)
                            eng.tensor_add(wsum[k], wsum[k], e[k4])
                        if final:
                            # wsum[k] is complete: issue its out-matmuls right
                            # away so PE chases the softmax tail
                            emit_out_k(k, last=(k == ks[-1]))

                # out accumulation runs in two passes over mp2 (the query-pos
                # chunk): 2 PSUM banks per pass instead of 4, freeing banks
                # for the score tiles
                po = {}
                po_started = {}
                cur_pass = [0]

                def emit_out_k(k, last):
                    mp2 = cur_pass[0]
                    for nn in range(2):
                        if nn not in po:
                            po[nn] = ps_o.tile([P, 512], FP32, tag="ps_o",
                                               name=f"po_{b}_{mp2}_{nn}")
                            po_started[nn] = False
                        for rc in range(SC):
                            is_last = last and rc == SC - 1
                            nc.tensor.matmul(
                                po[nn],
                                wsum[k][:, rc, mp2 * P : (mp2 + 1) * P],
                                nat[k][:, rc, nn * 512 : (nn + 1) * 512],
                                start=(not po_started[nn]),
                                stop=is_last,
                            )
                            po_started[nn] = True

                def finish_pass(osb):
                    mp2 = cur_pass[0]
                    for nn in range(2):
                        nc.scalar.activation(
                            out=osb[:, mp2, nn * 512 : (nn + 1) * 512],
                            in_=po[nn],
                            func=AF.Copy,
                            scale=0.5,
                        )
                    po.clear()
                    po_started.clear()
                    cur_pass[0] += 1

                if q[0] is None:
                    emit_q(0)
                if q[1] is None:
                    emit_q(1)
                s_cur = emit_scores(0)
                for a in range(NOPT):
                    if a == 0 and carry["flush"] is not None:
                        # previous item's pass-1 osb copies + out DMA, now
                        # clear of the boundary-critical ACT/Pool queues
                        carry["flush"]()
                        carry["flush"] = None
                    if a == NOPT - 1 and b + 1 < BPC:
                        # cross-item q pipelining: the next item's q(0)/q(1)
                        # keep PE busy through this item's softmax tail, and
                        # their ACT copies land ahead of the exp backlog
                        emit_q(0, xs=carry["x"], bq=b + 1, qdst=carry["q"])
                        emit_q(1, xs=carry["x"], bq=b + 1, qdst=carry["q"])
                    emit_softmax(a, s_cur,
                                 tail_split=(a == NOPT - 1 and b == BPC - 1))
                    if a + 1 < NOPT:
                        qth = q_group_thunks(a + 2) if a + 2 < NOPT else ()
                        s_cur = emit_scores(a + 1, qth)
                    if b + 1 < BPC:
                        # prefetch the next item's option a: one swdge trigger
                        # per iteration so Pool's ssb copies aren't delayed by
                        # a block of triggers; transposes ride the sync ring
                        carry["nat"][a] = load_nat(b + 1, a)
                        carry["x"][a] = transpose_opt(b + 1, a, carry["nat"][a])
                    if a == NOPT - 2:
                        # wsum for the last option is complete (it never
                        # scores against itself): overlap its out-matmuls
                        # with the final softmax
                        emit_out_k(NOPT - 1, last=False)
                osb = op_.tile([P, SC, H], FP32, tag="osb", name=f"osb_{b}")
                finish_pass(osb)
                # second mp2 pass: all wsums are ready, runs straight through
                emit_out_k(NOPT - 1, last=False)
                for k in range(NOPT - 1):
                    emit_out_k(k, last=(k == NOPT - 2))

                def flush(osb=osb, b=b, po_snap=dict(po), mp2=cur_pass[0]):
                    for nn in range(2):
                        nc.scalar.activation(
                            out=osb[:, mp2, nn * 512 : (nn + 1) * 512],
                            in_=po_snap[nn],
                            func=AF.Copy,
                            scale=0.5,
                        )
                    nc.scalar.dma_start(
                        out=out_d.ap()[b].rearrange("(sc p) h -> p sc h", p=P),
                        in_=osb,
                    )

                if b + 1 < BPC:
                    carry["flush"] = flush
                else:
                    flush()

    nc.compile()
    return nc


def _get_nc(reps: int = 1, cfg: dict | None = None):
    key = f"nc{reps}-{sorted((cfg or {}).items())}"
    if key not in _CACHE:
        _CACHE[key] = _build_bass(reps, cfg)
    return _CACHE[key]


def kernel(**inputs) -> np.ndarray:
    from concourse.bass_utils import run_bass_kernel_spmd

    nc = _get_nc()
    opts = [np.ascontiguousarray(np.asarray(inputs[f"option{i + 1}"], dtype=np.float32))
            for i in range(NOPT)]
    W = np.ascontiguousarray(np.asarray(inputs["W"], dtype=np.float32))

    in_maps = []
    for c in range(NCORES):
        m = {f"option{i + 1}": opts[i][c * BPC : (c + 1) * BPC] for i in range(NOPT)}
        m["W"] = W
        in_maps.append(m)

    res = run_bass_kernel_spmd(nc, in_maps, list(range(NCORES)))
    out = np.concatenate([res.results[c]["out"] for c in range(NCORES)], axis=0)
    return np.asarray(out, dtype=np.float32)

